# revision 24
# baseline (speedup 1.0000x reference)
"""Trainium2 Bass kernel for ExpKernelModule (Hawkes positive-likelihood intensities).

out[b,i] = sum_{j<i} alpha[u,v]*beta[u,v]*exp(clip(-beta[u,v]*(t_i-t_j), -20, 0))
with u=ct[b,i], v=ct[b,j], alpha=softplus(log_alpha), beta=softplus(log_beta).

Device algorithm (one batch per core, data-parallel over B=8):
the exp argument  log(a*b) - beta*(t_i - t_j)  is a rank-64 bilinear form over
the (receiver, trigger) type one-hots:

  arg[i,j] = W1[v,i]*oh[v,j] + W2[v,i]*(t_j*oh[v,j])     (sum over v)
  W1[v,i] = C1[u_i,v] - B[u_i,v]*t_i,  W2[v,i] = B[u_i,v],  oh[v,j] = 1[ct_j==v]

Per 128-row tile, matmuls produce the full exp-argument block in PSUM; ScalarE
applies Exp with a fused accum_out row-sum. Row tile r only needs columns
[0, 128*(r+1)); the diagonal 128x128 block gets a -1e4 additive strict-lower
mask (VectorE) before Exp.

PE dtype: float16 (full rate, 1 cyc/col). Each fp32 operand is split into a
hi/lo fp16 pair (22 effective mantissa bits); per-operand errors scale with
term magnitude, and large-magnitude args are exactly the dead ones (exp ~ 0).
Two accumulating matmuls per chunk cover all hi/lo cross products:
  mm1 K=128: [W1h, W1l, W2h, W2l] x [oh, oh, th*oh, th*oh]
  mm2 K=64:  [W2h, W2l]           x [tl*oh, tl*oh]
Measured end-to-end error vs the fp32 reference: ~8e-6 absmax-relative.
Host prep is O(L*D) index gathers only.
"""

import numpy as np

B_, L, D, P = 8, 2048, 32, 128
NT = L // P  # row tiles per batch
MASK_NEG = -1.0e4
MM_DTYPE = "bfloat16"  # fp16 pairs: ~7e-6 err; "bfloat16" pairs: ~4e-4 err

_cached = {}


def _build_nc():
    import concourse.bass as bass  # noqa: F401
    import concourse.tile as tile
    from concourse import bacc, mybir

    f32 = mybir.dt.float32
    f16 = getattr(mybir.dt, MM_DTYPE)

    nc = bacc.Bacc("TRN2", target_bir_lowering=False, debug=False, num_devices=8)
    wa_d = nc.dram_tensor("wa", (4 * D, L), f16, kind="ExternalInput").ap()
    ra_d = nc.dram_tensor("ra", (4 * D, L), f16, kind="ExternalInput").ap()
    wb_d = nc.dram_tensor("wb", (2 * D, L), f16, kind="ExternalInput").ap()
    rb_d = nc.dram_tensor("rb", (2 * D, L), f16, kind="ExternalInput").ap()
    m_d = nc.dram_tensor("m", (P, P), f32, kind="ExternalInput").ap()
    # out[p, r] = row-sum for global row i = 128*r + p; one contiguous DMA
    o_d = nc.dram_tensor("o", (P, NT), f32, kind="ExternalOutput").ap()

    with tile.TileContext(nc) as tc:
        with (
            tc.tile_pool(name="singles", bufs=1) as singles,
            tc.tile_pool(name="psum_v5", bufs=2, space="PSUM") as psum,
            tc.tile_pool(name="acc", bufs=4) as accp,
        ):
            # Interleave input DMAs in consumption order (512-col pieces),
            # spread across two HWDGE queues (sync + vector) for issue overlap.
            wa_sb = singles.tile([4 * D, L], f16)
            wb_sb = singles.tile([2 * D, L], f16)
            ra_sb = singles.tile([4 * D, L], f16)
            rb_sb = singles.tile([2 * D, L], f16)
            m_sb = singles.tile([P, P], f32)
            for c0 in range(0, L, 512):
                sl = slice(c0, c0 + 512)
                nc.sync.dma_start(ra_sb[:, sl], ra_d[:, sl])
                nc.scalar.dma_start(wa_sb[:, sl], wa_d[:, sl])
                nc.sync.dma_start(rb_sb[:, sl], rb_d[:, sl])
                nc.scalar.dma_start(wb_sb[:, sl], wb_d[:, sl])
                if c0 == 0:
                    nc.scalar.dma_start(m_sb[:, :], m_d[:, :])

            # HAM warm-up: junk matmuls on the mask tile while input DMAs are
            # in flight, so the PE clock is at 8/8 when the real stream starts.
            m16 = m_sb[:, :].bitcast(f16)  # (128, 256) finite junk
            for _ in range(3):
                jp = psum.tile([P, L], f32, tag="pt")
                for c0 in range(0, 2048, 512):
                    nc.tensor.matmul(
                        jp[:, c0 : c0 + 256], m16[:, 0:P], m16, start=True, stop=True
                    )

            acc = accp.tile([P, NT], f32)
            for rt in range(NT):
                ncols = P * (rt + 1)
                pt = psum.tile([P, L], f32)
                wsl = slice(rt * P, (rt + 1) * P)
                # all mm1 chunks first, then all mm2 chunks: consecutive PE
                # matmuls hit different PSUM banks, so fill overlaps drain
                # (same-bank accumulate pairs back-to-back serialize the PE).
                for c0 in range(0, ncols, 512):
                    w_len = min(512, ncols - c0)
                    csl = slice(c0, c0 + w_len)
                    nc.tensor.matmul(
                        pt[:, csl], wa_sb[:, wsl], ra_sb[:, csl],
                        start=True, stop=False,
                    )
                for c0 in range(0, ncols, 512):
                    w_len = min(512, ncols - c0)
                    csl = slice(c0, c0 + w_len)
                    nc.tensor.matmul(
                        pt[:, csl], wb_sb[:, wsl], rb_sb[:, csl],
                        start=False, stop=True,
                    )
                # strict-lower mask on the diagonal 128x128 block
                nc.vector.tensor_add(
                    pt[:, ncols - P : ncols], pt[:, ncols - P : ncols], m_sb[:, :]
                )
                nc.scalar.activation(
                    pt[:, :ncols],
                    pt[:, :ncols],
                    mybir.ActivationFunctionType.Exp,
                    accum_out=acc[:, rt : rt + 1],
                )
            nc.sync.dma_start(o_d[:, :], acc[:, :])

    nc.compile()
    return nc


def _softplus(x):
    return np.log1p(np.exp(-np.abs(x))) + np.maximum(x, 0.0)


def _host_prep(time_points, event_types, log_alpha, log_beta):
    t = np.asarray(time_points).astype(np.float64)  # (B, L)
    u = np.asarray(event_types).astype(np.int64)  # (B, L)
    A = _softplus(np.asarray(log_alpha).astype(np.float64))
    Bt = _softplus(np.asarray(log_beta).astype(np.float64))
    C1 = np.log(A * Bt)  # (D, D)

    if MM_DTYPE == "float16":
        f16 = np.float16
    else:
        import ml_dtypes

        f16 = ml_dtypes.bfloat16
    W1 = np.transpose(C1[u], (0, 2, 1)) - np.transpose(Bt[u], (0, 2, 1)) * t[:, None, :]
    W2 = np.transpose(Bt[u], (0, 2, 1))  # (B, D, L)
    W1h = W1.astype(f16); W1l = (W1 - W1h.astype(np.float64)).astype(f16)
    W2h = W2.astype(f16); W2l = (W2 - W2h.astype(np.float64)).astype(f16)
    th = t.astype(f16); tl = (t - th.astype(np.float64)).astype(f16)
    oh = (u[:, None, :] == np.arange(D)[None, :, None])  # (B, D, L) bool

    WA = np.concatenate([W1h, W1l, W2h, W2l], axis=1)  # (B, 4D, L) f16
    RA = np.concatenate(
        [oh, oh,
         th.astype(np.float64)[:, None, :] * oh,
         th.astype(np.float64)[:, None, :] * oh], axis=1
    ).astype(f16)  # (B, 4D, L)
    WB = np.concatenate([W2h, W2l], axis=1)  # (B, 2D, L)
    tlo = tl.astype(np.float64)[:, None, :] * oh
    RB = np.concatenate([tlo, tlo], axis=1).astype(f16)  # (B, 2D, L)
    mask = np.triu(np.full((P, P), MASK_NEG, dtype=np.float32), k=0)
    return WA, RA, WB, RB, mask


def _run(inputs, trace=False):
    from concourse.bass_utils import run_bass_kernel_spmd

    WA, RA, WB, RB, mask = _host_prep(
        inputs["time_points"],
        inputs["event_types"],
        inputs["log_alpha"],
        inputs["log_beta"],
    )
    if "nc" not in _cached:
        _cached["nc"] = _build_nc()
    nc = _cached["nc"]

    in_maps = [
        {"wa": WA[b], "ra": RA[b], "wb": WB[b], "rb": RB[b], "m": mask}
        for b in range(B_)
    ]
    bres = run_bass_kernel_spmd(
        nc, in_maps, core_ids=list(range(B_)), trace=trace,
        trace_cores=[0] if trace else None,
    )
    # o is (P, NT) with out[i=128*r+p] = o[p, r]
    out = np.stack(
        [bres.results[b]["o"].reshape(P, NT).T.reshape(L) for b in range(B_)], axis=0
    )
    return out.astype(np.float32), bres


def kernel(**inputs) -> np.ndarray:
    out, _ = _run(inputs, trace=False)
    return out


# revision 25
# speedup vs baseline: 1.0552x; 1.0552x over previous
"""Trainium2 Bass kernel for ExpKernelModule (Hawkes positive-likelihood intensities).

out[b,i] = sum_{j<i} alpha[u,v]*beta[u,v]*exp(clip(-beta[u,v]*(t_i-t_j), -20, 0))
with u=ct[b,i], v=ct[b,j], alpha=softplus(log_alpha), beta=softplus(log_beta).

Device algorithm (one batch per core, data-parallel over B=8):
the exp argument  log(a*b) - beta*(t_i - t_j)  is a rank-64 bilinear form over
the (receiver, trigger) type one-hots:

  arg[i,j] = W1[v,i]*oh[v,j] + W2[v,i]*(t_j*oh[v,j])     (sum over v)
  W1[v,i] = C1[u_i,v] - B[u_i,v]*t_i,  W2[v,i] = B[u_i,v],  oh[v,j] = 1[ct_j==v]

Per 128-row tile, matmuls produce the full exp-argument block in PSUM; ScalarE
applies Exp with a fused accum_out row-sum. Row tile r only needs columns
[0, 128*(r+1)); the diagonal 128x128 block gets a -1e4 additive strict-lower
mask (VectorE) before Exp.

PE dtype: float16 (full rate, 1 cyc/col). Each fp32 operand is split into a
hi/lo fp16 pair (22 effective mantissa bits); per-operand errors scale with
term magnitude, and large-magnitude args are exactly the dead ones (exp ~ 0).
Two accumulating matmuls per chunk cover all hi/lo cross products:
  mm1 K=128: [W1h, W1l, W2h, W2l] x [oh, oh, th*oh, th*oh]
  mm2 K=64:  [W2h, W2l]           x [tl*oh, tl*oh]
Measured end-to-end error vs the fp32 reference: ~8e-6 absmax-relative.
Host prep is O(L*D) index gathers only.
"""

import numpy as np

B_, L, D, P = 8, 2048, 32, 128
NT = L // P  # row tiles per batch
MASK_NEG = -1.0e4
MM_DTYPE = "float16"  # fp16 pairs: ~7e-6 err; "bfloat16" pairs: ~4e-4 err

_cached = {}


def _build_nc():
    import concourse.bass as bass  # noqa: F401
    import concourse.tile as tile
    from concourse import bacc, mybir

    f32 = mybir.dt.float32
    f16 = getattr(mybir.dt, MM_DTYPE)

    nc = bacc.Bacc("TRN2", target_bir_lowering=False, debug=False, num_devices=8)
    wa_d = nc.dram_tensor("wa", (4 * D, L), f16, kind="ExternalInput").ap()
    ra_d = nc.dram_tensor("ra", (4 * D, L), f16, kind="ExternalInput").ap()
    wb_d = nc.dram_tensor("wb", (2 * D, L), f16, kind="ExternalInput").ap()
    rb_d = nc.dram_tensor("rb", (2 * D, L), f16, kind="ExternalInput").ap()
    m_d = nc.dram_tensor("m", (P, P), f32, kind="ExternalInput").ap()
    # out[p, r] = row-sum for global row i = 128*r + p; one contiguous DMA
    o_d = nc.dram_tensor("o", (P, NT), f32, kind="ExternalOutput").ap()

    with tile.TileContext(nc) as tc:
        with (
            tc.tile_pool(name="singles", bufs=1) as singles,
            tc.tile_pool(name="psum_v5", bufs=2, space="PSUM") as psum,
            tc.tile_pool(name="acc", bufs=4) as accp,
        ):
            # Interleave input DMAs in consumption order (512-col pieces),
            # spread across two HWDGE queues (sync + vector) for issue overlap.
            wa_sb = singles.tile([4 * D, L], f16)
            wb_sb = singles.tile([2 * D, L], f16)
            ra_sb = singles.tile([4 * D, L], f16)
            rb_sb = singles.tile([2 * D, L], f16)
            m_sb = singles.tile([P, P], f32)
            for c0 in range(0, L, 512):
                sl = slice(c0, c0 + 512)
                nc.sync.dma_start(ra_sb[:, sl], ra_d[:, sl])
                nc.scalar.dma_start(wa_sb[:, sl], wa_d[:, sl])
                nc.sync.dma_start(rb_sb[:, sl], rb_d[:, sl])
                nc.scalar.dma_start(wb_sb[:, sl], wb_d[:, sl])
                if c0 == 0:
                    nc.scalar.dma_start(m_sb[:, :], m_d[:, :])

            acc = accp.tile([P, NT], f32)
            for rt in range(NT):
                ncols = P * (rt + 1)
                pt = psum.tile([P, L], f32)
                wsl = slice(rt * P, (rt + 1) * P)
                # all mm1 chunks first, then all mm2 chunks: consecutive PE
                # matmuls hit different PSUM banks, so fill overlaps drain
                # (same-bank accumulate pairs back-to-back serialize the PE).
                for c0 in range(0, ncols, 512):
                    w_len = min(512, ncols - c0)
                    csl = slice(c0, c0 + w_len)
                    nc.tensor.matmul(
                        pt[:, csl], wa_sb[:, wsl], ra_sb[:, csl],
                        start=True, stop=False,
                    )
                for c0 in range(0, ncols, 512):
                    w_len = min(512, ncols - c0)
                    csl = slice(c0, c0 + w_len)
                    nc.tensor.matmul(
                        pt[:, csl], wb_sb[:, wsl], rb_sb[:, csl],
                        start=False, stop=True,
                    )
                # strict-lower mask on the diagonal 128x128 block
                nc.vector.tensor_add(
                    pt[:, ncols - P : ncols], pt[:, ncols - P : ncols], m_sb[:, :]
                )
                nc.scalar.activation(
                    pt[:, :ncols],
                    pt[:, :ncols],
                    mybir.ActivationFunctionType.Exp,
                    accum_out=acc[:, rt : rt + 1],
                )
            nc.sync.dma_start(o_d[:, :], acc[:, :])

    nc.compile()
    return nc


def _softplus(x):
    return np.log1p(np.exp(-np.abs(x))) + np.maximum(x, 0.0)


def _host_prep(time_points, event_types, log_alpha, log_beta):
    t = np.asarray(time_points).astype(np.float64)  # (B, L)
    u = np.asarray(event_types).astype(np.int64)  # (B, L)
    A = _softplus(np.asarray(log_alpha).astype(np.float64))
    Bt = _softplus(np.asarray(log_beta).astype(np.float64))
    C1 = np.log(A * Bt)  # (D, D)

    if MM_DTYPE == "float16":
        f16 = np.float16
    else:
        import ml_dtypes

        f16 = ml_dtypes.bfloat16
    W1 = np.transpose(C1[u], (0, 2, 1)) - np.transpose(Bt[u], (0, 2, 1)) * t[:, None, :]
    W2 = np.transpose(Bt[u], (0, 2, 1))  # (B, D, L)
    W1h = W1.astype(f16); W1l = (W1 - W1h.astype(np.float64)).astype(f16)
    W2h = W2.astype(f16); W2l = (W2 - W2h.astype(np.float64)).astype(f16)
    th = t.astype(f16); tl = (t - th.astype(np.float64)).astype(f16)
    oh = (u[:, None, :] == np.arange(D)[None, :, None])  # (B, D, L) bool

    WA = np.concatenate([W1h, W1l, W2h, W2l], axis=1)  # (B, 4D, L) f16
    RA = np.concatenate(
        [oh, oh,
         th.astype(np.float64)[:, None, :] * oh,
         th.astype(np.float64)[:, None, :] * oh], axis=1
    ).astype(f16)  # (B, 4D, L)
    WB = np.concatenate([W2h, W2l], axis=1)  # (B, 2D, L)
    tlo = tl.astype(np.float64)[:, None, :] * oh
    RB = np.concatenate([tlo, tlo], axis=1).astype(f16)  # (B, 2D, L)
    mask = np.triu(np.full((P, P), MASK_NEG, dtype=np.float32), k=0)
    return WA, RA, WB, RB, mask


def _run(inputs, trace=False):
    from concourse.bass_utils import run_bass_kernel_spmd

    WA, RA, WB, RB, mask = _host_prep(
        inputs["time_points"],
        inputs["event_types"],
        inputs["log_alpha"],
        inputs["log_beta"],
    )
    if "nc" not in _cached:
        _cached["nc"] = _build_nc()
    nc = _cached["nc"]

    in_maps = [
        {"wa": WA[b], "ra": RA[b], "wb": WB[b], "rb": RB[b], "m": mask}
        for b in range(B_)
    ]
    bres = run_bass_kernel_spmd(
        nc, in_maps, core_ids=list(range(B_)), trace=trace,
        trace_cores=[0] if trace else None,
    )
    # o is (P, NT) with out[i=128*r+p] = o[p, r]
    out = np.stack(
        [bres.results[b]["o"].reshape(P, NT).T.reshape(L) for b in range(B_)], axis=0
    )
    return out.astype(np.float32), bres


def kernel(**inputs) -> np.ndarray:
    out, _ = _run(inputs, trace=False)
    return out


# revision 27
# speedup vs baseline: 1.1752x; 1.1138x over previous
"""Trainium2 Bass kernel for ExpKernelModule (Hawkes positive-likelihood intensities).

out[b,i] = sum_{j<i} alpha[u,v]*beta[u,v]*exp(clip(-beta[u,v]*(t_i-t_j), -20, 0))
with u=ct[b,i], v=ct[b,j], alpha=softplus(log_alpha), beta=softplus(log_beta).

Device algorithm (one batch per core, data-parallel over B=8):
the exp argument  log(a*b) - beta*(t_i - t_j)  is a rank-64 bilinear form over
the (receiver, trigger) type one-hots:

  arg[i,j] = W1[v,i]*oh[v,j] + W2[v,i]*(t_j*oh[v,j])     (sum over v)
  W1[v,i] = C1[u_i,v] - B[u_i,v]*t_i,  W2[v,i] = B[u_i,v],  oh[v,j] = 1[ct_j==v]

Per 128-row tile, matmuls produce the full exp-argument block in PSUM; ScalarE
applies Exp with a fused accum_out row-sum. Row tile r only needs columns
[0, 128*(r+1)); the diagonal 128x128 block gets a -1e4 additive strict-lower
mask (VectorE) before Exp.

PE dtype: float16. Each fp32 operand is split into a hi/lo fp16 pair (22
effective mantissa bits); per-operand errors scale with term magnitude, and
large-magnitude args are exactly the dead ones (exp ~ 0). Two accumulating
matmuls per chunk cover all needed hi/lo cross products:
  mm1 K=128: [W1h, W1l, W2h, W2l] x [oh, oh, th*oh, th*oh]
  mm2 K=64:  [W2h, W2l]           x [tl*oh, tl*oh]
(Measured on HW: fp32-PSUM-accumulating matmuls retire at ~2 cyc/col for
bf16/fp16 alike, so fp16 costs the same as bf16 here and keeps fp32-level
accuracy. fp32 runs at 4 cyc/col and fp32r ~2x, both slower or less exact.)
Measured end-to-end error vs the fp32 reference: ~7e-6 absmax-relative.
Host prep is O(L*D) index gathers only.
"""

import numpy as np

B_, L, D, P = 8, 2048, 32, 128
NT = L // P  # row tiles per batch
MASK_NEG = -1.0e4
MM_DTYPE = "float16"  # fp16 pairs: ~7e-6 err; "bfloat16" pairs: ~4e-4 err

_cached = {}


def _build_nc():
    import concourse.bass as bass  # noqa: F401
    import concourse.tile as tile
    from concourse import bacc, mybir

    f32 = mybir.dt.float32
    f16 = getattr(mybir.dt, MM_DTYPE)

    nc = bacc.Bacc("TRN2", target_bir_lowering=False, debug=False, num_devices=8)
    wa_d = nc.dram_tensor("wa", (4 * D, L), f16, kind="ExternalInput").ap()
    ra_d = nc.dram_tensor("ra", (4 * D, L), f16, kind="ExternalInput").ap()
    wb_d = nc.dram_tensor("wb", (2 * D, L), f16, kind="ExternalInput").ap()
    rb_d = nc.dram_tensor("rb", (2 * D, L), f16, kind="ExternalInput").ap()
    m_d = nc.dram_tensor("m", (P, P), f32, kind="ExternalInput").ap()
    # out[p, r] = row-sum for global row i = 128*r + p; one contiguous DMA
    o_d = nc.dram_tensor("o", (P, NT), f32, kind="ExternalOutput").ap()

    with tile.TileContext(nc) as tc:
        with (
            tc.tile_pool(name="singles", bufs=1) as singles,
            tc.tile_pool(name="psum_v5", bufs=2, space="PSUM") as psum,
            tc.tile_pool(name="acc", bufs=4) as accp,
        ):
            # Interleave input DMAs in consumption order (512-col pieces),
            # spread across the two HWDGE queues (sync + scalar) for overlap.
            wa_sb = singles.tile([4 * D, L], f16)
            wb_sb = singles.tile([2 * D, L], f16)
            ra_sb = singles.tile([4 * D, L], f16)
            rb_sb = singles.tile([2 * D, L], f16)
            m_sb = singles.tile([P, P], f32)
            for c0 in range(0, L, 512):
                sl = slice(c0, c0 + 512)
                nc.sync.dma_start(ra_sb[:, sl], ra_d[:, sl])
                nc.scalar.dma_start(wa_sb[:, sl], wa_d[:, sl])
                nc.sync.dma_start(rb_sb[:, sl], rb_d[:, sl])
                nc.scalar.dma_start(wb_sb[:, sl], wb_d[:, sl])
                if c0 == 0:
                    nc.scalar.dma_start(m_sb[:, :], m_d[:, :])

            acc = accp.tile([P, NT], f32)
            for rt in range(NT):
                ncols = P * (rt + 1)
                pt = psum.tile([P, L], f32)
                wsl = slice(rt * P, (rt + 1) * P)
                # all mm1 chunks first, then all mm2 chunks: consecutive PE
                # matmuls hit different PSUM banks, so fill overlaps drain
                # (same-bank accumulate pairs back-to-back serialize the PE).
                for c0 in range(0, ncols, 512):
                    w_len = min(512, ncols - c0)
                    csl = slice(c0, c0 + w_len)
                    nc.tensor.matmul(
                        pt[:, csl], wa_sb[:, wsl], ra_sb[:, csl],
                        start=True, stop=False,
                    )
                for c0 in range(0, ncols, 512):
                    w_len = min(512, ncols - c0)
                    csl = slice(c0, c0 + w_len)
                    nc.tensor.matmul(
                        pt[:, csl], wb_sb[:, wsl], rb_sb[:, csl],
                        start=False, stop=True,
                    )
                # strict-lower mask on the diagonal 128x128 block
                nc.vector.tensor_add(
                    pt[:, ncols - P : ncols], pt[:, ncols - P : ncols], m_sb[:, :]
                )
                nc.scalar.activation(
                    pt[:, :ncols],
                    pt[:, :ncols],
                    mybir.ActivationFunctionType.Exp,
                    accum_out=acc[:, rt : rt + 1],
                )
            nc.sync.dma_start(o_d[:, :], acc[:, :])

    nc.compile()
    return nc


def _softplus(x):
    return np.log1p(np.exp(-np.abs(x))) + np.maximum(x, 0.0)


def _host_prep(time_points, event_types, log_alpha, log_beta):
    t = np.asarray(time_points).astype(np.float64)  # (B, L)
    u = np.asarray(event_types).astype(np.int64)  # (B, L)
    A = _softplus(np.asarray(log_alpha).astype(np.float64))
    Bt = _softplus(np.asarray(log_beta).astype(np.float64))
    C1 = np.log(A * Bt)  # (D, D)

    if MM_DTYPE == "float16":
        f16 = np.float16
    else:
        import ml_dtypes

        f16 = ml_dtypes.bfloat16
    W1 = np.transpose(C1[u], (0, 2, 1)) - np.transpose(Bt[u], (0, 2, 1)) * t[:, None, :]
    W2 = np.transpose(Bt[u], (0, 2, 1))  # (B, D, L)
    W1h = W1.astype(f16); W1l = (W1 - W1h.astype(np.float64)).astype(f16)
    W2h = W2.astype(f16); W2l = (W2 - W2h.astype(np.float64)).astype(f16)
    th = t.astype(f16); tl = (t - th.astype(np.float64)).astype(f16)
    oh = (u[:, None, :] == np.arange(D)[None, :, None])  # (B, D, L) bool

    WA = np.concatenate([W1h, W1l, W2h, W2l], axis=1)  # (B, 4D, L) f16
    RA = np.concatenate(
        [oh, oh,
         th.astype(np.float64)[:, None, :] * oh,
         th.astype(np.float64)[:, None, :] * oh], axis=1
    ).astype(f16)  # (B, 4D, L)
    WB = np.concatenate([W2h, W2l], axis=1)  # (B, 2D, L)
    tlo = tl.astype(np.float64)[:, None, :] * oh
    RB = np.concatenate([tlo, tlo], axis=1).astype(f16)  # (B, 2D, L)
    mask = np.triu(np.full((P, P), MASK_NEG, dtype=np.float32), k=0)
    return WA, RA, WB, RB, mask


def _run(inputs, trace=False):
    from concourse.bass_utils import run_bass_kernel_spmd

    WA, RA, WB, RB, mask = _host_prep(
        inputs["time_points"],
        inputs["event_types"],
        inputs["log_alpha"],
        inputs["log_beta"],
    )
    if "nc" not in _cached:
        _cached["nc"] = _build_nc()
    nc = _cached["nc"]

    in_maps = [
        {"wa": WA[b], "ra": RA[b], "wb": WB[b], "rb": RB[b], "m": mask}
        for b in range(B_)
    ]
    bres = run_bass_kernel_spmd(
        nc, in_maps, core_ids=list(range(B_)), trace=trace,
        trace_cores=[0] if trace else None,
    )
    # o is (P, NT) with out[i=128*r+p] = o[p, r]
    out = np.stack(
        [bres.results[b]["o"].reshape(P, NT).T.reshape(L) for b in range(B_)], axis=0
    )
    return out.astype(np.float32), bres


def kernel(**inputs) -> np.ndarray:
    out, _ = _run(inputs, trace=False)
    return out
